# revision 33
# baseline (speedup 1.0000x reference)
"""Trainium2 Bass kernel for nn_Attention_48498770706573.

Fused QKV-projection + masked softmax attention, sharded over 8 NeuronCores:
data-parallel over batch (B=2), tensor-parallel over heads (16 -> 4 per
core). Each core computes its (batch, 4-head) shard end to end; the host
only slices/transposes inputs (layout only, no arithmetic) and concatenates
the disjoint output shards.

Per-core dataflow (all "transposed-land"):
  qT/kT/vT [D, N] fp32 DRAM declared float32r, converted to bf16 on-chip,
  projections (bf16 matmuls, fp32 PSUM) -> qtT/ktT [256, N] and an
  ones-augmented vt_aug [128, t, head, [v+bv | ones]] (bv folded via a K=1
  rank-1 matmul: the PV numerator P@(v+bv) = PV + bv x rowsum comes out of
  one matmul),
  scores S^T[nk, nq] as bf16 matmuls with two heads row-packed (K=64 each),
  exp on ACT straight out of PSUM -> bf16 (1/32 scale folded in),
  bool mask cast u8->bf16 via SWDGE cast-DMA, then one broadcast DVE
  multiply (2x bf16 mode) per score group,
  PV per head: one M=128 matmul with stationary [vt_h+bv | ones]: PSUM rows
  0-63 = numerator, rows 64-127 = rowsum replicas (no separate rowsum
  matmuls), all 4 heads in one 4-bank PSUM tile,
  tail: 1/rowsum via ACT exp(-ln(rs)) on the 4-bank replica region (both
  funcs steered into one ACT table set), partition remap 64->0 via one
  SBUF->SBUF DMA, one fused [64, 4, 512] DVE multiply, head-major output.
  The PV stream lags scores by 4 k-tiles; the q-projection for chunk c+2 is
  emitted at each chunk boundary so the PE stays busy through the tail.
"""

import os

import numpy as np

import concourse.bacc as bacc
import concourse.hw_specs as _hw_specs
import concourse.mybir as mybir
import concourse.tile as tile
from concourse.bass_utils import run_bass_kernel_spmd

# The kernel uses both Exp and Ln. The act-table-load placement pass picks,
# per activation, the first act_info.json set containing the function —
# landing Exp in "exp_and_others" and Ln in "natural_log", which thrashes
# the ACT table RAM (~2.7us per switch) at every chunk tail. Steer both to
# the combined "natural_log_exp_and_others" set by hiding Exp/Ln from the
# other sets (dict order — and hence act_func_set_id — is kept).
_orig_get_act_tables = _hw_specs.get_activation_tables


def _patched_get_act_tables(module_arch):
    exp = mybir.ActivationFunctionType.Exp
    ln = mybir.ActivationFunctionType.Ln
    out = {}
    for name, funcs in _orig_get_act_tables(module_arch).items():
        f = set(funcs)
        if name != "natural_log_exp_and_others":
            f.discard(exp)
            f.discard(ln)
        out[name] = f
    return out


_hw_specs.get_activation_tables = _patched_get_act_tables
bacc.get_activation_tables = _patched_get_act_tables

B, NQ, NK, D, H = 2, 2048, 2048, 1024, 16
DH = D // H  # 64
N_CORES = 8
HPC = H // (N_CORES // B)  # heads per core = 4
JW = HPC * DH  # per-core projection width = 256
NKT = NK // 128  # 16 nk tiles
NCH = 4  # nq chunks
CHW = NQ // NCH  # 512
DT = 8  # contraction d-tiles

f32 = mybir.dt.float32
f32r = mybir.dt.float32r
bf16 = mybir.dt.bfloat16
u8 = mybir.dt.uint8


def _build():
    nc = bacc.Bacc(
        "TRN2", target_bir_lowering=False, debug=False, num_devices=N_CORES
    )

    qT = nc.dram_tensor("qT", [D, NQ], f32r, kind="ExternalInput")
    kT = nc.dram_tensor("kT", [D, NK], f32r, kind="ExternalInput")
    vT = nc.dram_tensor("vT", [D, NK], f32r, kind="ExternalInput")
    maskT = nc.dram_tensor("maskT", [NK, NQ], u8, kind="ExternalInput")
    wqT = nc.dram_tensor("wqT", [D, JW], f32r, kind="ExternalInput")
    wkT = nc.dram_tensor("wkT", [D, JW], f32r, kind="ExternalInput")
    wvT = nc.dram_tensor("wvT", [D, JW], f32r, kind="ExternalInput")
    bqd = nc.dram_tensor("bq", [2, 128], f32, kind="ExternalInput")
    bkd = nc.dram_tensor("bk", [2, 128], f32, kind="ExternalInput")
    bvrowd = nc.dram_tensor("bvrow", [1, JW], bf16, kind="ExternalInput")
    ones1d = nc.dram_tensor("ones1", [1, 128], bf16, kind="ExternalInput")
    # head-major output: row h*64+e, col q
    o = nc.dram_tensor("o", [2 * 128, NQ], f32, kind="ExternalOutput")

    with tile.TileContext(nc) as tc:
        with (
            tc.tile_pool(name="consts", bufs=1) as consts,
            tc.tile_pool(name="wtmp", bufs=1) as wtmp,
            tc.tile_pool(name="stage", bufs=12) as stage,
            tc.tile_pool(name="vbfp", bufs=8) as vbfp,
            tc.tile_pool(name="xbfp", bufs=10) as xbfp,
            tc.tile_pool(name="qpool", bufs=12) as qpool,
            tc.tile_pool(name="m8pool", bufs=16) as m8pool,
            tc.tile_pool(name="mbpool", bufs=3) as mbpool,
            tc.tile_pool(name="projout", bufs=1) as projout,
            tc.tile_pool(name="ppool", bufs=12) as ppool,
            tc.tile_pool(name="invp", bufs=1) as invp,
            tc.tile_pool(name="outsb", bufs=1) as outsb,
            tc.tile_pool(name="sps", bufs=2, space="PSUM") as sps,
            tc.tile_pool(name="pvps", bufs=1, space="PSUM") as pvps,
        ):
            # ---- constants ----
            w_sb = {}

            def dma_w(name, dram):
                t = wtmp.tile([128, DT, JW], f32r, tag=f"wt{name}", name="wt")
                for d in range(DT):
                    nc.sync.dma_start(t[:, d], dram[d * 128 : (d + 1) * 128, :])
                return t

            def conv_w(name, t):
                wb = consts.tile([128, DT, JW], bf16, tag=f"w{name}", name="w")
                for d in range(DT):
                    nc.vector.tensor_copy(wb[:, d], t[:, d])
                w_sb[name] = wb

            # ---- decoupled input DMAs (emitted in priority order) ----
            def dma_x_chunk(src, ch, tiles=None, pool=None):
                pool = pool or stage
                tiles = {} if tiles is None else tiles
                for d in range(DT):
                    x = pool.tile([128, CHW], f32r, tag="xc", name="x")
                    nc.sync.dma_start(
                        x,
                        src[d * 128 : (d + 1) * 128, ch * CHW : (ch + 1) * CHW],
                    )
                    tiles[(d, ch)] = x
                return tiles

            bq_sb = consts.tile([128, 2], f32, tag="bq")
            bk_sb = consts.tile([128, 2], f32, tag="bk")
            for m in range(2):
                nc.sync.dma_start(
                    bq_sb[:, m : m + 1],
                    bqd[m : m + 1, :].rearrange("a b -> b a"),
                )
                nc.sync.dma_start(
                    bk_sb[:, m : m + 1],
                    bkd[m : m + 1, :].rearrange("a b -> b a"),
                )
            bvrow_sb = consts.tile([1, JW], bf16, tag="bvrow")
            nc.sync.dma_start(bvrow_sb, bvrowd[:])
            ones1_sb = consts.tile([1, 128], bf16, tag="ones1")
            nc.sync.dma_start(ones1_sb, ones1d[:])
            wtk = dma_w("k", wkT)
            conv_w("k", wtk)
            k_tiles = {}
            for d in range(DT):
                for ch in range(NCH):
                    x = stage.tile([128, CHW], f32r, tag="xc", name="x")
                    nc.sync.dma_start(
                        x, kT[d * 128 : (d + 1) * 128, ch * CHW : (ch + 1) * CHW]
                    )
                    k_tiles[(d, ch)] = x
            wtq = dma_w("q", wqT)
            wtv = dma_w("v", wvT)
            m8 = []
            for t in range(NKT):
                mt8 = m8pool.tile([128, NQ], u8, tag="m8", name="m8")
                nc.sync.dma_start(mt8, maskT[t * 128 : (t + 1) * 128, :])
                m8.append(mt8)
            q_tiles = dma_x_chunk(qT, 0)
            q_later = {}
            dma_x_chunk(qT, 1, q_later, pool=qpool)
            v_tiles = {}
            for ch in range(NCH):
                dma_x_chunk(vT, ch, v_tiles)
            for ch in range(2, NCH):
                dma_x_chunk(qT, ch, q_later, pool=qpool)

            # ---- projections ----
            qtT = projout.tile([128, 2, NQ], bf16, tag="qtT")
            ktT = projout.tile([128, 2, NK], bf16, tag="ktT")
            # per (t, head): [vt_h + bv | ones] -> one M=128 PV stationary
            vt_aug = projout.tile([128, NKT, HPC, 128], bf16, tag="vt")
            nc.vector.memset(
                vt_aug.rearrange("p t h (a c) -> p (t h) a c", a=2)[:, :, 1, :],
                1.0,
            )

            def proj_qk_full(name, tiles, dst, bias):
                """All 4 chunks; m0 accumulates into two 2-bank sps tiles,
                m1 into the 4 bank-slices of the pvps tile."""
                ps0t = [
                    sps.tile([128, 2 * CHW], f32, tag="s", name=f"ps0{i}")
                    for i in range(2)
                ]
                ps0 = [
                    ps0t[0][:, 0:CHW],
                    ps0t[0][:, CHW:],
                    ps0t[1][:, 0:CHW],
                    ps0t[1][:, CHW:],
                ]
                pv = pvps.tile([128, HPC, CHW], f32, tag="pv", name="pspv")
                ps1 = [pv[:, i, :] for i in range(HPC)]
                for d in range(DT):
                    xb = {}
                    for ch in range(NCH):
                        xb[ch] = xbfp.tile(
                            [128, CHW], bf16, tag="xb", name="xb"
                        )
                        nc.vector.tensor_copy(xb[ch], tiles[(d, ch)])
                    for ch in range(NCH):
                        nc.tensor.matmul(
                            ps0[ch],
                            w_sb[name][:, d, 0:128],
                            xb[ch],
                            start=(d == 0),
                            stop=(d == DT - 1),
                        )
                    for ch in range(NCH):
                        nc.tensor.matmul(
                            ps1[ch],
                            w_sb[name][:, d, 128:256],
                            xb[ch],
                            start=(d == 0),
                            stop=(d == DT - 1),
                        )
                for ch2 in range(2):
                    nc.vector.tensor_scalar_add(
                        dst[:, 0, ch2 * 2 * CHW : (ch2 + 1) * 2 * CHW],
                        ps0t[ch2],
                        bias[:, 0:1],
                    )
                for ch in range(NCH):
                    nc.vector.tensor_scalar_add(
                        dst[:, 1, ch * CHW : (ch + 1) * CHW],
                        ps1[ch],
                        bias[:, 1:2],
                    )

            def proj_qk_chunk(name, tiles, ch, dst, bias, cast_dma=False):
                xb = {}
                for d in range(DT):
                    xb[d] = xbfp.tile([128, CHW], bf16, tag="xb", name="xb")
                    if cast_dma:
                        nc.gpsimd.dma_start(xb[d], tiles[(d, ch)])
                    else:
                        nc.vector.tensor_copy(xb[d], tiles[(d, ch)])
                ps = sps.tile([128, 2 * CHW], f32, tag="s", name="pps")
                for m in range(2):
                    psm = ps[:, m * CHW : (m + 1) * CHW]
                    for d in range(DT):
                        nc.tensor.matmul(
                            psm,
                            w_sb[name][:, d, m * 128 : (m + 1) * 128],
                            xb[d],
                            start=(d == 0),
                            stop=(d == DT - 1),
                        )
                for m in range(2):
                    nc.vector.tensor_scalar_add(
                        dst[:, m, ch * CHW : (ch + 1) * CHW],
                        ps[:, m * CHW : (m + 1) * CHW],
                        bias[:, m : m + 1],
                    )

            def proj_v():
                # bf16 x-tiles so the weight loads pipeline with the matmuls
                vbf = {}
                pv = pvps.tile([128, HPC, CHW], f32, tag="pv", name="pspv")
                for n in range(NKT):
                    ch, nn_ = divmod(n, 4)
                    ps = pv[:, n % 4, 0:JW]
                    for d in range(DT):
                        if (d, ch) not in vbf:
                            xb = vbfp.tile(
                                [128, CHW], bf16, tag="vb", name="vb"
                            )
                            nc.vector.tensor_copy(xb, v_tiles[(d, ch)])
                            vbf[(d, ch)] = xb
                        nc.tensor.matmul(
                            ps,
                            vbf[(d, ch)][:, nn_ * 128 : (nn_ + 1) * 128],
                            w_sb["v"][:, d, :],
                            start=(d == 0),
                            stop=False,
                        )
                    # += ones^T x bvrow : folds the V bias into vt so the
                    # PV matmul yields PV + bv (x) rowsum directly
                    nc.tensor.matmul(
                        ps, ones1_sb, bvrow_sb, start=False, stop=True
                    )
                    nc.vector.tensor_copy(
                        vt_aug[:, n, :, 0:64],
                        ps.rearrange("p (h c) -> p h c", h=HPC),
                    )

            # ---- attention ----
            def scores_group(pair, t, cs, p_tiles):
                sp = sps.tile([128, 2 * CHW], f32, tag="s", name="sp")
                for hh in range(2):
                    nc.tensor.matmul(
                        sp[:, hh * CHW : (hh + 1) * CHW],
                        ktT[
                            64 * hh : 64 * (hh + 1),
                            pair,
                            t * 128 : (t + 1) * 128,
                        ],
                        qtT[64 * hh : 64 * (hh + 1), pair, cs],
                        start=True,
                        stop=True,
                    )
                p = ppool.tile([128, 2 * CHW], bf16, tag="p", name="p")
                nc.scalar.activation(
                    out=p,
                    in_=sp,
                    func=mybir.ActivationFunctionType.Exp,
                    scale=1.0 / 32.0,
                )
                if pair == 0:
                    mb = mbpool.tile([128, CHW], bf16, tag="mb", name="mb")
                    # SWDGE cast DMA u8 -> bf16 (frees GpSimd compute)
                    nc.gpsimd.dma_start(mb, m8[t][:, cs])
                    p_tiles[("mb", t)] = mb
                else:
                    mb = p_tiles[("mb", t)]
                p3 = p.rearrange("p (h c) -> p h c", h=2)
                nc.vector.tensor_mul(
                    p3,
                    p3,
                    mb.rearrange("p (a c) -> p a c", a=1).to_broadcast(
                        (128, 2, CHW)
                    ),
                )
                p_tiles[(pair, t)] = p

            def pv_t(t, p_tiles, pv):
                st, sp_ = t == 0, t == NKT - 1
                for h in range(HPC):
                    pair, hh = divmod(h, 2)
                    p = p_tiles[(pair, t)]
                    nc.tensor.matmul(
                        pv[:, h, :],
                        vt_aug[:, t, h, :],
                        p[:, hh * CHW : (hh + 1) * CHW],
                        start=st,
                        stop=sp_,
                    )

            def chunk_tail(cs, pv, split=False):
                # inv = exp(-ln(rs)) on the rowsum-replica rows (eps is
                # negligible vs rs, which is >= hundreds)
                rsr = pv[64:128]  # [64, HPC, CHW]
                nc.scalar.activation(
                    out=rsr, in_=rsr, func=mybir.ActivationFunctionType.Ln
                )
                iv = invp.tile([128, HPC, CHW], bf16, tag="iv", name="iv")
                nc.scalar.activation(
                    out=iv[64:128],
                    in_=rsr,
                    func=mybir.ActivationFunctionType.Exp,
                    scale=-1.0,
                )
                # partition remap 64..127 -> 0..63 (same tile, disjoint rows)
                nc.sync.dma_start(iv[0:64], iv[64:128])
                osb = outsb.tile([128, HPC, CHW], f32, tag="o", name="osb")
                nc.vector.tensor_mul(osb[0:64], pv[0:64], iv[0:64])
                nc.sync.dma_start(
                    o.rearrange("(h e) q -> e h q", h=HPC)[:, :, cs],
                    osb[0:64],
                )

            proj_qk_full("k", k_tiles, ktT, bk_sb)
            conv_w("q", wtq)
            conv_w("v", wtv)
            proj_qk_chunk("q", q_tiles, 0, qtT, bq_sb)

            LAG = 4
            # chunk 0's first score groups are emitted before proj_v so the
            # exp/mask pipeline spins up while V is still being projected
            # (pv_t(0) depends only on vt_aug's n=0 slice)
            p_tiles0 = {}
            cs0 = slice(0, CHW)
            for t in range(LAG):
                for pair in range(2):
                    scores_group(pair, t, cs0, p_tiles0)
            proj_v()
            # chunk 1's q-projection before the attention loop; chunk c+2's
            # is emitted at each chunk boundary (PE filler during the tail)
            proj_qk_chunk("q", q_later, 1, qtT, bq_sb, cast_dma=True)

            for ch in range(NCH):
                cs = slice(ch * CHW, (ch + 1) * CHW)
                p_tiles = p_tiles0 if ch == 0 else {}
                pv = pvps.tile([128, HPC, CHW], f32, tag="pv", name="pv")
                for t in range(LAG if ch == 0 else 0, NKT + LAG):
                    if t < NKT:
                        for pair in range(2):
                            scores_group(pair, t, cs, p_tiles)
                    # PV batched in pairs of k-tiles: pv->pv paces at the
                    # streaming floor while sc<->pv switches cost ~790ns
                    if t >= LAG + 1 and (t - LAG) % 2 == 1:
                        pv_t(t - LAG - 1, p_tiles, pv)
                        pv_t(t - LAG, p_tiles, pv)
                if ch + 2 < NCH:
                    proj_qk_chunk(
                        "q", q_later, ch + 2, qtT, bq_sb, cast_dma=True
                    )
                chunk_tail(cs, pv, split=(ch == NCH - 1))

    nc.compile()
    return nc


_NC = None


def _get_nc():
    global _NC
    if _NC is None:
        _NC = _build()
    return _NC


def _shard(inputs):
    import ml_dtypes

    q, k, v = inputs["q"], inputs["k"], inputs["v"]
    mask = inputs["mask"]
    Wq, bq, Wk, bk, Wv, bv = (
        inputs[n] for n in ("Wq", "bq", "Wk", "bk", "Wv", "bv")
    )
    qT = [np.ascontiguousarray(np.asarray(q[b], np.float32).T) for b in range(B)]
    kT = [np.ascontiguousarray(np.asarray(k[b], np.float32).T) for b in range(B)]
    vT = [np.ascontiguousarray(np.asarray(v[b], np.float32).T) for b in range(B)]
    mT = [
        np.ascontiguousarray(np.asarray(mask[b]).T).view(np.uint8)
        for b in range(B)
    ]
    ones1 = np.ones((1, 128), ml_dtypes.bfloat16)
    in_maps = []
    for c in range(N_CORES):
        b, jg = divmod(c, N_CORES // B)
        j0 = jg * JW
        in_maps.append(
            {
                "qT": qT[b],
                "kT": kT[b],
                "vT": vT[b],
                "maskT": mT[b],
                "wqT": np.ascontiguousarray(
                    np.asarray(Wq, np.float32)[j0 : j0 + JW, :].T
                ),
                "wkT": np.ascontiguousarray(
                    np.asarray(Wk, np.float32)[j0 : j0 + JW, :].T
                ),
                "wvT": np.ascontiguousarray(
                    np.asarray(Wv, np.float32)[j0 : j0 + JW, :].T
                ),
                "bq": np.asarray(bq, np.float32)[j0 : j0 + JW].reshape(2, 128),
                "bk": np.asarray(bk, np.float32)[j0 : j0 + JW].reshape(2, 128),
                "bvrow": np.asarray(bv, np.float32)[j0 : j0 + JW]
                .reshape(1, JW)
                .astype(ml_dtypes.bfloat16),
                "ones1": ones1,
            }
        )
    return in_maps


LAST_RESULT = None


def kernel(**inputs) -> np.ndarray:
    global LAST_RESULT
    nc = _get_nc()
    in_maps = _shard(inputs)
    trace = bool(int(os.environ.get("KTRACE", "0")))
    res = run_bass_kernel_spmd(
        nc,
        in_maps,
        core_ids=list(range(N_CORES)),
        trace=trace,
        trace_cores=[0] if trace else None,
    )
    LAST_RESULT = res
    out = np.empty((B, NQ, D), np.float32)
    for c in range(N_CORES):
        b, jg = divmod(c, N_CORES // B)
        j0 = jg * JW
        oc = res.results[c]["o"]  # [256, NQ] head-major
        out[b, :, j0 : j0 + JW] = (
            oc.reshape(HPC, DH, NQ).transpose(2, 0, 1).reshape(NQ, JW)
        )
    return out


if __name__ == "__main__":
    if os.environ.get("KBUILD_ONLY"):
        import tempfile

        from concourse.bass_utils import compile_bass_kernel

        nc = _build()
        with tempfile.TemporaryDirectory() as td:
            compile_bass_kernel(nc, td)
        print("BUILD+COMPILE OK")


# revision 34
# speedup vs baseline: 1.1971x; 1.1971x over previous
"""Trainium2 Bass kernel for nn_Attention_48498770706573.

Fused QKV-projection + masked softmax attention, sharded over 8 NeuronCores:
data-parallel over batch (B=2), tensor-parallel over heads (16 -> 4 per
core). Each core computes its (batch, 4-head) shard end to end; the host
only slices/transposes inputs (layout only, no arithmetic) and concatenates
the disjoint output shards.

Per-core dataflow (all "transposed-land"):
  qT/kT/vT [D, N] fp32 DRAM declared float32r, converted to bf16 on-chip,
  projections (bf16 matmuls, fp32 PSUM) -> qtT/ktT [256, N] and an
  ones-augmented vt_aug [128, t, head, [v+bv | ones]] (bv folded via a K=1
  rank-1 matmul: the PV numerator P@(v+bv) = PV + bv x rowsum comes out of
  one matmul),
  scores S^T[nk, nq] as bf16 matmuls with two heads row-packed (K=64 each),
  exp on ACT straight out of PSUM -> bf16 (1/32 scale folded in),
  bool mask cast u8->bf16 via SWDGE cast-DMA, then one broadcast DVE
  multiply (2x bf16 mode) per score group,
  PV per head: one M=128 matmul with stationary [vt_h+bv | ones]: PSUM rows
  0-63 = numerator, rows 64-127 = rowsum replicas (no separate rowsum
  matmuls), all 4 heads in one 4-bank PSUM tile,
  tail: 1/rowsum via ACT exp(-ln(rs)) on the 4-bank replica region (both
  funcs steered into one ACT table set), partition remap 64->0 via one
  SBUF->SBUF DMA, one fused [64, 4, 512] DVE multiply, head-major output.
  The PV stream lags scores by 4 k-tiles; the q-projection for chunk c+2 is
  emitted at each chunk boundary so the PE stays busy through the tail.
"""

import os

import numpy as np

import concourse.bacc as bacc
import concourse.hw_specs as _hw_specs
import concourse.mybir as mybir
import concourse.tile as tile
from concourse.bass_utils import run_bass_kernel_spmd

# The kernel uses both Exp and Ln. The act-table-load placement pass picks,
# per activation, the first act_info.json set containing the function —
# landing Exp in "exp_and_others" and Ln in "natural_log", which thrashes
# the ACT table RAM (~2.7us per switch) at every chunk tail. Steer both to
# the combined "natural_log_exp_and_others" set by hiding Exp/Ln from the
# other sets (dict order — and hence act_func_set_id — is kept).
_orig_get_act_tables = _hw_specs.get_activation_tables


def _patched_get_act_tables(module_arch):
    exp = mybir.ActivationFunctionType.Exp
    ln = mybir.ActivationFunctionType.Ln
    out = {}
    for name, funcs in _orig_get_act_tables(module_arch).items():
        f = set(funcs)
        if name != "natural_log_exp_and_others":
            f.discard(exp)
            f.discard(ln)
        out[name] = f
    return out


_hw_specs.get_activation_tables = _patched_get_act_tables
bacc.get_activation_tables = _patched_get_act_tables

B, NQ, NK, D, H = 2, 2048, 2048, 1024, 16
DH = D // H  # 64
N_CORES = 8
HPC = H // (N_CORES // B)  # heads per core = 4
JW = HPC * DH  # per-core projection width = 256
NKT = NK // 128  # 16 nk tiles
NCH = 4  # nq chunks
CHW = NQ // NCH  # 512
DT = 8  # contraction d-tiles

f32 = mybir.dt.float32
f32r = mybir.dt.float32r
bf16 = mybir.dt.bfloat16
u8 = mybir.dt.uint8


def _build():
    nc = bacc.Bacc(
        "TRN2", target_bir_lowering=False, debug=False, num_devices=N_CORES
    )

    qT = nc.dram_tensor("qT", [D, NQ], f32r, kind="ExternalInput")
    kT = nc.dram_tensor("kT", [D, NK], f32r, kind="ExternalInput")
    vT = nc.dram_tensor("vT", [D, NK], f32r, kind="ExternalInput")
    maskT = nc.dram_tensor("maskT", [NK, NQ], u8, kind="ExternalInput")
    wqT = nc.dram_tensor("wqT", [D, JW], f32r, kind="ExternalInput")
    wkT = nc.dram_tensor("wkT", [D, JW], f32r, kind="ExternalInput")
    wvT = nc.dram_tensor("wvT", [D, JW], f32r, kind="ExternalInput")
    bqd = nc.dram_tensor("bq", [2, 128], f32, kind="ExternalInput")
    bkd = nc.dram_tensor("bk", [2, 128], f32, kind="ExternalInput")
    bvrowd = nc.dram_tensor("bvrow", [1, JW], bf16, kind="ExternalInput")
    ones1d = nc.dram_tensor("ones1", [1, 128], bf16, kind="ExternalInput")
    # head-major output: row h*64+e, col q
    o = nc.dram_tensor("o", [2 * 128, NQ], f32, kind="ExternalOutput")

    with tile.TileContext(nc) as tc:
        with (
            tc.tile_pool(name="consts", bufs=1) as consts,
            tc.tile_pool(name="wtmp", bufs=1) as wtmp,
            tc.tile_pool(name="stage", bufs=12) as stage,
            tc.tile_pool(name="vbfp", bufs=8) as vbfp,
            tc.tile_pool(name="xbfp", bufs=10) as xbfp,
            tc.tile_pool(name="qpool", bufs=12) as qpool,
            tc.tile_pool(name="m8pool", bufs=16) as m8pool,
            tc.tile_pool(name="mbpool", bufs=3) as mbpool,
            tc.tile_pool(name="projout", bufs=1) as projout,
            tc.tile_pool(name="ppool", bufs=12) as ppool,
            tc.tile_pool(name="invp", bufs=1) as invp,
            tc.tile_pool(name="outsb", bufs=1) as outsb,
            tc.tile_pool(name="sps", bufs=2, space="PSUM") as sps,
            tc.tile_pool(name="pvps", bufs=1, space="PSUM") as pvps,
        ):
            # ---- constants ----
            w_sb = {}

            def dma_w(name, dram):
                t = wtmp.tile([128, DT, JW], f32r, tag=f"wt{name}", name="wt")
                for d in range(DT):
                    nc.sync.dma_start(t[:, d], dram[d * 128 : (d + 1) * 128, :])
                return t

            def conv_w(name, t):
                wb = consts.tile([128, DT, JW], bf16, tag=f"w{name}", name="w")
                for d in range(DT):
                    nc.vector.tensor_copy(wb[:, d], t[:, d])
                w_sb[name] = wb

            # ---- decoupled input DMAs (emitted in priority order) ----
            def dma_x_chunk(src, ch, tiles=None, pool=None):
                pool = pool or stage
                tiles = {} if tiles is None else tiles
                for d in range(DT):
                    x = pool.tile([128, CHW], f32r, tag="xc", name="x")
                    nc.sync.dma_start(
                        x,
                        src[d * 128 : (d + 1) * 128, ch * CHW : (ch + 1) * CHW],
                    )
                    tiles[(d, ch)] = x
                return tiles

            wtk = wtmp.tile([128, DT, JW], f32r, tag="wtk", name="wt")
            k_tiles = {}
            for d in range(DT):
                nc.sync.dma_start(wtk[:, d], wkT[d * 128 : (d + 1) * 128, :])
                for ch in range(NCH):
                    x = stage.tile([128, CHW], f32r, tag="xc", name="x")
                    nc.sync.dma_start(
                        x, kT[d * 128 : (d + 1) * 128, ch * CHW : (ch + 1) * CHW]
                    )
                    k_tiles[(d, ch)] = x
            conv_w("k", wtk)
            wtq = dma_w("q", wqT)
            wtv = dma_w("v", wvT)
            bq_sb = consts.tile([128, 2], f32, tag="bq")
            bk_sb = consts.tile([128, 2], f32, tag="bk")
            for m in range(2):
                nc.sync.dma_start(
                    bq_sb[:, m : m + 1],
                    bqd[m : m + 1, :].rearrange("a b -> b a"),
                )
                nc.sync.dma_start(
                    bk_sb[:, m : m + 1],
                    bkd[m : m + 1, :].rearrange("a b -> b a"),
                )
            bvrow_sb = consts.tile([1, JW], bf16, tag="bvrow")
            nc.sync.dma_start(bvrow_sb, bvrowd[:])
            ones1_sb = consts.tile([1, 128], bf16, tag="ones1")
            nc.sync.dma_start(ones1_sb, ones1d[:])
            m8 = []
            for t in range(NKT):
                mt8 = m8pool.tile([128, NQ], u8, tag="m8", name="m8")
                # SWDGE queue: issues in parallel with the Sync queue
                nc.gpsimd.dma_start(mt8, maskT[t * 128 : (t + 1) * 128, :])
                m8.append(mt8)
            q_tiles = dma_x_chunk(qT, 0)
            q_later = {}
            dma_x_chunk(qT, 1, q_later, pool=qpool)
            v_tiles = {}
            for ch in range(NCH):
                dma_x_chunk(vT, ch, v_tiles)
            for ch in range(2, NCH):
                dma_x_chunk(qT, ch, q_later, pool=qpool)

            # ---- projections ----
            qtT = projout.tile([128, 2, NQ], bf16, tag="qtT")
            ktT = projout.tile([128, 2, NK], bf16, tag="ktT")
            # per (t, head): [vt_h + bv | ones] -> one M=128 PV stationary
            vt_aug = projout.tile([128, NKT, HPC, 128], bf16, tag="vt")
            nc.vector.memset(
                vt_aug.rearrange("p t h (a c) -> p (t h) a c", a=2)[:, :, 1, :],
                1.0,
            )

            def proj_qk_full(name, tiles, dst, bias):
                """All 4 chunks; m0 accumulates into two 2-bank sps tiles,
                m1 into the 4 bank-slices of the pvps tile."""
                ps0t = [
                    sps.tile([128, 2 * CHW], f32, tag="s", name=f"ps0{i}")
                    for i in range(2)
                ]
                ps0 = [
                    ps0t[0][:, 0:CHW],
                    ps0t[0][:, CHW:],
                    ps0t[1][:, 0:CHW],
                    ps0t[1][:, CHW:],
                ]
                pv = pvps.tile([128, HPC, CHW], f32, tag="pv", name="pspv")
                ps1 = [pv[:, i, :] for i in range(HPC)]
                for d in range(DT):
                    xb = {}
                    for ch in range(NCH):
                        xb[ch] = xbfp.tile(
                            [128, CHW], bf16, tag="xb", name="xb"
                        )
                        nc.vector.tensor_copy(xb[ch], tiles[(d, ch)])
                    for ch in range(NCH):
                        nc.tensor.matmul(
                            ps0[ch],
                            w_sb[name][:, d, 0:128],
                            xb[ch],
                            start=(d == 0),
                            stop=(d == DT - 1),
                        )
                    for ch in range(NCH):
                        nc.tensor.matmul(
                            ps1[ch],
                            w_sb[name][:, d, 128:256],
                            xb[ch],
                            start=(d == 0),
                            stop=(d == DT - 1),
                        )
                for ch2 in range(2):
                    nc.vector.tensor_scalar_add(
                        dst[:, 0, ch2 * 2 * CHW : (ch2 + 1) * 2 * CHW],
                        ps0t[ch2],
                        bias[:, 0:1],
                    )
                for ch in range(NCH):
                    nc.vector.tensor_scalar_add(
                        dst[:, 1, ch * CHW : (ch + 1) * CHW],
                        ps1[ch],
                        bias[:, 1:2],
                    )

            def proj_qk_chunk(name, tiles, ch, dst, bias, cast_dma=False):
                xb = {}
                for d in range(DT):
                    xb[d] = xbfp.tile([128, CHW], bf16, tag="xb", name="xb")
                    if cast_dma:
                        nc.gpsimd.dma_start(xb[d], tiles[(d, ch)])
                    else:
                        nc.vector.tensor_copy(xb[d], tiles[(d, ch)])
                ps = sps.tile([128, 2 * CHW], f32, tag="s", name="pps")
                for m in range(2):
                    psm = ps[:, m * CHW : (m + 1) * CHW]
                    for d in range(DT):
                        nc.tensor.matmul(
                            psm,
                            w_sb[name][:, d, m * 128 : (m + 1) * 128],
                            xb[d],
                            start=(d == 0),
                            stop=(d == DT - 1),
                        )
                for m in range(2):
                    nc.vector.tensor_scalar_add(
                        dst[:, m, ch * CHW : (ch + 1) * CHW],
                        ps[:, m * CHW : (m + 1) * CHW],
                        bias[:, m : m + 1],
                    )

            def proj_v():
                # bf16 x-tiles so the weight loads pipeline with the matmuls
                vbf = {}
                pv = pvps.tile([128, HPC, CHW], f32, tag="pv", name="pspv")
                for n in range(NKT):
                    ch, nn_ = divmod(n, 4)
                    ps = pv[:, n % 4, 0:JW]
                    for d in range(DT):
                        if (d, ch) not in vbf:
                            xb = vbfp.tile(
                                [128, CHW], bf16, tag="vb", name="vb"
                            )
                            nc.vector.tensor_copy(xb, v_tiles[(d, ch)])
                            vbf[(d, ch)] = xb
                        nc.tensor.matmul(
                            ps,
                            vbf[(d, ch)][:, nn_ * 128 : (nn_ + 1) * 128],
                            w_sb["v"][:, d, :],
                            start=(d == 0),
                            stop=False,
                        )
                    # += ones^T x bvrow : folds the V bias into vt so the
                    # PV matmul yields PV + bv (x) rowsum directly
                    nc.tensor.matmul(
                        ps, ones1_sb, bvrow_sb, start=False, stop=True
                    )
                    nc.vector.tensor_copy(
                        vt_aug[:, n, :, 0:64],
                        ps.rearrange("p (h c) -> p h c", h=HPC),
                    )

            # ---- attention ----
            def scores_group(pair, t, cs, p_tiles):
                sp = sps.tile([128, 2 * CHW], f32, tag="s", name="sp")
                for hh in range(2):
                    nc.tensor.matmul(
                        sp[:, hh * CHW : (hh + 1) * CHW],
                        ktT[
                            64 * hh : 64 * (hh + 1),
                            pair,
                            t * 128 : (t + 1) * 128,
                        ],
                        qtT[64 * hh : 64 * (hh + 1), pair, cs],
                        start=True,
                        stop=True,
                    )
                p = ppool.tile([128, 2 * CHW], bf16, tag="p", name="p")
                nc.scalar.activation(
                    out=p,
                    in_=sp,
                    func=mybir.ActivationFunctionType.Exp,
                    scale=1.0 / 32.0,
                )
                if pair == 0:
                    mb = mbpool.tile([128, CHW], bf16, tag="mb", name="mb")
                    # SWDGE cast DMA u8 -> bf16 (frees GpSimd compute)
                    nc.gpsimd.dma_start(mb, m8[t][:, cs])
                    p_tiles[("mb", t)] = mb
                else:
                    mb = p_tiles[("mb", t)]
                p3 = p.rearrange("p (h c) -> p h c", h=2)
                nc.vector.tensor_mul(
                    p3,
                    p3,
                    mb.rearrange("p (a c) -> p a c", a=1).to_broadcast(
                        (128, 2, CHW)
                    ),
                )
                p_tiles[(pair, t)] = p

            def pv_t(t, p_tiles, pv):
                st, sp_ = t == 0, t == NKT - 1
                for h in range(HPC):
                    pair, hh = divmod(h, 2)
                    p = p_tiles[(pair, t)]
                    nc.tensor.matmul(
                        pv[:, h, :],
                        vt_aug[:, t, h, :],
                        p[:, hh * CHW : (hh + 1) * CHW],
                        start=st,
                        stop=sp_,
                    )

            def chunk_tail(cs, pv, split=False):
                # inv = exp(-ln(rs)) on the rowsum-replica rows (eps is
                # negligible vs rs, which is >= hundreds)
                rsr = pv[64:128]  # [64, HPC, CHW]
                nc.scalar.activation(
                    out=rsr, in_=rsr, func=mybir.ActivationFunctionType.Ln
                )
                iv = invp.tile([128, HPC, CHW], bf16, tag="iv", name="iv")
                nc.scalar.activation(
                    out=iv[64:128],
                    in_=rsr,
                    func=mybir.ActivationFunctionType.Exp,
                    scale=-1.0,
                )
                # partition remap 64..127 -> 0..63 (same tile, disjoint rows)
                nc.sync.dma_start(iv[0:64], iv[64:128])
                osb = outsb.tile([128, HPC, CHW], f32, tag="o", name="osb")
                nc.vector.tensor_mul(osb[0:64], pv[0:64], iv[0:64])
                nc.sync.dma_start(
                    o.rearrange("(h e) q -> e h q", h=HPC)[:, :, cs],
                    osb[0:64],
                )

            proj_qk_full("k", k_tiles, ktT, bk_sb)
            conv_w("q", wtq)
            conv_w("v", wtv)
            proj_qk_chunk("q", q_tiles, 0, qtT, bq_sb)

            LAG = 4
            # chunk 0's first score groups are emitted before proj_v so the
            # exp/mask pipeline spins up while V is still being projected
            # (pv_t(0) depends only on vt_aug's n=0 slice)
            p_tiles0 = {}
            cs0 = slice(0, CHW)
            for t in range(LAG):
                for pair in range(2):
                    scores_group(pair, t, cs0, p_tiles0)
            proj_v()
            # chunk 1's q-projection before the attention loop; chunk c+2's
            # is emitted at each chunk boundary (PE filler during the tail)
            proj_qk_chunk("q", q_later, 1, qtT, bq_sb, cast_dma=True)

            for ch in range(NCH):
                cs = slice(ch * CHW, (ch + 1) * CHW)
                p_tiles = p_tiles0 if ch == 0 else {}
                pv = pvps.tile([128, HPC, CHW], f32, tag="pv", name="pv")
                for t in range(LAG if ch == 0 else 0, NKT + LAG):
                    if t < NKT:
                        for pair in range(2):
                            scores_group(pair, t, cs, p_tiles)
                    # PV batched in pairs of k-tiles: pv->pv paces at the
                    # streaming floor while sc<->pv switches cost ~790ns
                    if t >= LAG + 1 and (t - LAG) % 2 == 1:
                        pv_t(t - LAG - 1, p_tiles, pv)
                        pv_t(t - LAG, p_tiles, pv)
                if ch + 2 < NCH:
                    proj_qk_chunk(
                        "q", q_later, ch + 2, qtT, bq_sb, cast_dma=True
                    )
                chunk_tail(cs, pv, split=(ch == NCH - 1))

    nc.compile()
    return nc


_NC = None


def _get_nc():
    global _NC
    if _NC is None:
        _NC = _build()
    return _NC


def _shard(inputs):
    import ml_dtypes

    q, k, v = inputs["q"], inputs["k"], inputs["v"]
    mask = inputs["mask"]
    Wq, bq, Wk, bk, Wv, bv = (
        inputs[n] for n in ("Wq", "bq", "Wk", "bk", "Wv", "bv")
    )
    qT = [np.ascontiguousarray(np.asarray(q[b], np.float32).T) for b in range(B)]
    kT = [np.ascontiguousarray(np.asarray(k[b], np.float32).T) for b in range(B)]
    vT = [np.ascontiguousarray(np.asarray(v[b], np.float32).T) for b in range(B)]
    mT = [
        np.ascontiguousarray(np.asarray(mask[b]).T).view(np.uint8)
        for b in range(B)
    ]
    ones1 = np.ones((1, 128), ml_dtypes.bfloat16)
    in_maps = []
    for c in range(N_CORES):
        b, jg = divmod(c, N_CORES // B)
        j0 = jg * JW
        in_maps.append(
            {
                "qT": qT[b],
                "kT": kT[b],
                "vT": vT[b],
                "maskT": mT[b],
                "wqT": np.ascontiguousarray(
                    np.asarray(Wq, np.float32)[j0 : j0 + JW, :].T
                ),
                "wkT": np.ascontiguousarray(
                    np.asarray(Wk, np.float32)[j0 : j0 + JW, :].T
                ),
                "wvT": np.ascontiguousarray(
                    np.asarray(Wv, np.float32)[j0 : j0 + JW, :].T
                ),
                "bq": np.asarray(bq, np.float32)[j0 : j0 + JW].reshape(2, 128),
                "bk": np.asarray(bk, np.float32)[j0 : j0 + JW].reshape(2, 128),
                "bvrow": np.asarray(bv, np.float32)[j0 : j0 + JW]
                .reshape(1, JW)
                .astype(ml_dtypes.bfloat16),
                "ones1": ones1,
            }
        )
    return in_maps


LAST_RESULT = None


def kernel(**inputs) -> np.ndarray:
    global LAST_RESULT
    nc = _get_nc()
    in_maps = _shard(inputs)
    trace = bool(int(os.environ.get("KTRACE", "0")))
    res = run_bass_kernel_spmd(
        nc,
        in_maps,
        core_ids=list(range(N_CORES)),
        trace=trace,
        trace_cores=[0] if trace else None,
    )
    LAST_RESULT = res
    out = np.empty((B, NQ, D), np.float32)
    for c in range(N_CORES):
        b, jg = divmod(c, N_CORES // B)
        j0 = jg * JW
        oc = res.results[c]["o"]  # [256, NQ] head-major
        out[b, :, j0 : j0 + JW] = (
            oc.reshape(HPC, DH, NQ).transpose(2, 0, 1).reshape(NQ, JW)
        )
    return out


if __name__ == "__main__":
    if os.environ.get("KBUILD_ONLY"):
        import tempfile

        from concourse.bass_utils import compile_bass_kernel

        nc = _build()
        with tempfile.TemporaryDirectory() as td:
            compile_bass_kernel(nc, td)
        print("BUILD+COMPILE OK")
